# revision 2
# baseline (speedup 1.0000x reference)
"""Trainium2 Bass kernel v2 for nn_ContextEncoderLayer.

Key redesign vs v1 (which was DVE-bound at ~38us per big f32 DVE op on this HW):
  - scores via PE "query pullback": qk[m,(l,h)] = sum_dh Wk[m,(h,dh)] q[l,(h,dh)],
    then scores[row,(l,h)] = sum_m target[row,m] qk[m,(l,h)] -- all matmuls.
    Block-diagonal validity mask, attn mask and k-bias folded into the PE
    accumulation as tiny extra matmuls, so exp() batches 8 tiles per ACT op.
  - V path via associativity: ctx = (P . target) @ Wv.  tmixT[m,(l,h)] comes out
    of PE transpose-ready; evacuated in [128,2048] ACT ops.
  - All PSUM evacuation on ACT (scalar) engine; DVE only does bf16 2x-mode ops
    and tiny [128,1] scalars (f32 DVE ops >=[128,512] are pathologically slow).
  - LN affine g1/beta1 folded into W1/b1 host-side; bv folded into src; beta1+b2
    folded into the ff2 bias row.
"""

import sys

sys.path.insert(0, "/opt/trn_rl_repo")

from contextlib import ExitStack

import numpy as np
import ml_dtypes

import concourse.bacc as bacc
import concourse.tile as tile
from concourse import mybir
from concourse.bass_utils import run_bass_kernel_spmd
from concourse.masks import make_identity

import os
DBG = bool(os.environ.get("K2DBG"))
L, D, DM, H, FF = 2048, 32, 1024, 16, 4096
DH = DM // H  # 64
SCALE = float(np.sqrt(DH))  # 8.0
NCORES = 8
LC = L // NCORES  # 256 positions per core
NT = LC * D // 128  # 64 row tiles per core (128 rows = 4 pos x 32 cands)
NLT = LC // 128  # 2 l-tiles
NC_DM = DM // 128  # 8
CH = NC_DM + 1  # 9 (ones row for bias folds)
NFF = FF // 128  # 32
TPB = NT // NLT  # 32 tiles per l-tile
BF = mybir.dt.bfloat16
F32 = mybir.dt.float32
NEG = -1.0e30

_CACHE = {}


def _consts():
    a4 = np.zeros((4, 128), np.float32)
    for r in range(128):
        a4[r // 32, r] = 1.0
    bneg = np.full((4, 64), NEG, np.float32)
    for i in range(4):
        for h in range(H):
            bneg[i, i * 16 + h] = 0.0
    ucol = np.zeros((128, 32 * 32), np.float32)
    for j in range(32):
        ucol[:, j * 32 + j] = 1.0
    selp = np.zeros((32, 4 * 128), np.float32)
    for t in range(32):
        for lp in range(4):
            selp[t, lp * 128 + 4 * t + lp] = 1.0
    ones = np.ones((1, 256), np.float32)
    zrow = np.zeros((1, 512), np.float32)
    bf = ml_dtypes.bfloat16
    return (a4.astype(bf), bneg.astype(bf), ucol.astype(bf), ones.astype(bf),
            zrow.astype(bf), selp)


def _build_nc(repeat=1):
    nc = bacc.Bacc("TRN2", target_bir_lowering=False, debug=False, num_devices=NCORES)

    xt_in = nc.dram_tensor("xt", [NT, 128, CH * 128], BF, kind="ExternalInput")
    tgt_in = nc.dram_tensor("tgt", [NT, 128, DM], BF, kind="ExternalInput")
    st_in = nc.dram_tensor("st", [128, CH * LC], BF, kind="ExternalInput")
    wq_in = nc.dram_tensor("wq", [128, CH * 1024], BF, kind="ExternalInput")
    wkz_in = nc.dram_tensor("wkz", [128, NC_DM * H * 128], BF, kind="ExternalInput")
    bkz_in = nc.dram_tensor("bkz", [128, H], BF, kind="ExternalInput")
    wv_in = nc.dram_tensor("wv", [128, NC_DM * 1024], BF, kind="ExternalInput")
    mkf_in = nc.dram_tensor("mkf", [1, NT * 128], BF, kind="ExternalInput")
    srcf_in = nc.dram_tensor("srcf", [LC, DM], BF, kind="ExternalInput")
    w1_in = nc.dram_tensor("w1p", [NFF, 128, NC_DM * 128], BF, kind="ExternalInput")
    b1r_in = nc.dram_tensor("b1r", [1, NFF * 128], BF, kind="ExternalInput")
    w2_in = nc.dram_tensor("w2p", [NFF, 128, DM], BF, kind="ExternalInput")
    b2r_in = nc.dram_tensor("b2r", [1, DM], BF, kind="ExternalInput")
    g1r_in = nc.dram_tensor("g1r", [128, DM], BF, kind="ExternalInput")
    g2r_in = nc.dram_tensor("g2r", [128, DM], BF, kind="ExternalInput")
    be2_in = nc.dram_tensor("be2r", [128, DM], BF, kind="ExternalInput")
    out = nc.dram_tensor("out", [LC, DM], F32, kind="ExternalOutput")
    if DBG:
        dbg_qt = nc.dram_tensor("dbg_qt", [128, 1024], F32, kind="ExternalOutput")
        dbg_qk = nc.dram_tensor("dbg_qk", [128, 2048], F32, kind="ExternalOutput")
        dbg_pm = nc.dram_tensor("dbg_pm", [128, 2048], F32, kind="ExternalOutput")
        dbg_rd = nc.dram_tensor("dbg_rd", [128, 16], F32, kind="ExternalOutput")
        dbg_x1 = nc.dram_tensor("dbg_x1", [128, 1024], F32, kind="ExternalOutput")
        dbg_xn = nc.dram_tensor("dbg_xn", [128, 1024], F32, kind="ExternalOutput")
        dbg_xt = nc.dram_tensor("dbg_xt", [128, 2048], F32, kind="ExternalOutput")
        dbg_f1 = nc.dram_tensor("dbg_f1", [128, 1024], F32, kind="ExternalOutput")

    a4_np, bneg_np, ucol_np, ones_np, zrow_np, selp_np = _consts()
    selp_c = nc.inline_tensor(selp_np, name="selpc")
    a4_c = nc.inline_tensor(a4_np, name="a4c")
    bneg_c = nc.inline_tensor(bneg_np, name="bnegc")
    ucol_c = nc.inline_tensor(ucol_np, name="ucolc")
    ones_c = nc.inline_tensor(ones_np, name="onesc")
    zrow_c = nc.inline_tensor(zrow_np, name="zrowc")

    AL = mybir.AluOpType
    AF = mybir.ActivationFunctionType

    with tile.TileContext(nc) as tc, ExitStack() as top:
        consts = top.enter_context(tc.tile_pool(name="consts", bufs=1))

        def cload(name, src, shape, dt=BF):
            t = consts.tile(shape, dt, name=name)
            nc.sync.dma_start(t[:], src[:])
            return t

        wkz_sb = cload("wkz_sb", wkz_in, [128, NC_DM * H * 128])
        bkz_sb = cload("bkz_sb", bkz_in, [128, H])
        wv_sb = cload("wv_sb", wv_in, [128, NC_DM * 1024])
        b1r_sb = cload("b1r_sb", b1r_in, [1, NFF * 128])
        b2r_sb = cload("b2r_sb", b2r_in, [1, DM])
        g1r_sb = cload("g1r_sb", g1r_in, [128, DM])
        g2r_sb = cload("g2r_sb", g2r_in, [128, DM])
        be2_sb = cload("be2_sb", be2_in, [128, DM])
        a4_sb = consts.tile([4, 128], BF, name="a4_sb")
        nc.sync.dma_start(a4_sb[:], a4_c[:])
        bneg_sb = consts.tile([4, 64], BF, name="bneg_sb")
        nc.sync.dma_start(bneg_sb[:], bneg_c[:])
        ucol_sb = consts.tile([128, 32 * 32], BF, name="ucol_sb")
        nc.sync.dma_start(ucol_sb[:], ucol_c[:])
        ones_sb = consts.tile([1, 256], BF, name="ones_sb")
        nc.sync.dma_start(ones_sb[:], ones_c[:])
        zrow_sb = consts.tile([1, 512], BF, name="zrow_sb")
        nc.sync.dma_start(zrow_sb[:], zrow_c[:])
        selp_sb = consts.tile([32, 4 * 128], F32, name="selp_sb")
        nc.sync.dma_start(selp_sb[:], selp_c[:])
        ident = consts.tile([128, 128], BF, name="ident")
        make_identity(nc, ident[:])
        eps_sb = consts.tile([128, 1], F32, name="eps_sb")
        nc.vector.memset(eps_sb[:], 1e-5)

        def zmm(ps_slice, n):
            # write zeros to [128, n] psum region (sets has_written -> later
            # matmuls with start=False accumulate safely in any order)
            nc.tensor.matmul(ps_slice, ones_sb[0:1, 0:128], zrow_sb[0:1, 0:n],
                             start=True, stop=False, skip_group_check=True)

        xres = top.enter_context(tc.tile_pool(name="xres", bufs=1))
        xtp = top.enter_context(tc.tile_pool(name="xtp", bufs=1))

        for rep in range(repeat):
            xn_tiles = []
            xT_sb = xtp.tile([128, NC_DM * LC], BF, name=f"xT{rep}", tag="xT")

            # ---------------- P1: qT = Wq^T srcT  (both l-tiles) -----------
            qT_sbs = []
            with tc.tile_pool(name="p1pool", bufs=1) as p1p, \
                 tc.tile_pool(name="qt_ps", bufs=1, space="PSUM") as qt_psp:
                st_sb = p1p.tile([128, CH * LC], BF, name=f"st{rep}", tag="st")
                nc.sync.dma_start(st_sb[:], st_in[:])
                wq_sb = p1p.tile([128, CH * 1024], BF, name=f"wqs{rep}", tag="wqs")
                nc.sync.dma_start(wq_sb[:], wq_in[:])
                for lt in range(NLT):
                    qt_ps = qt_psp.tile([128, 1024], F32, name=f"qtps{rep}_{lt}",
                                        tag=f"qtps{lt}")
                    zmm(qt_ps[:, 0:512], 512)
                    zmm(qt_ps[:, 512:1024], 512)
                    for o in range(8):
                        for c in range(CH):
                            pr = slice(0, 128) if c < NC_DM else slice(0, 1)
                            nc.tensor.matmul(
                                qt_ps[:, o * 128:(o + 1) * 128],
                                wq_sb[pr, c * 1024 + o * 128: c * 1024 + (o + 1) * 128],
                                st_sb[pr, c * LC + lt * 128: c * LC + (lt + 1) * 128],
                                start=False, stop=(c == CH - 1),
                                skip_group_check=True)
                    qT_sb = xres.tile([128, 1024], BF, name=f"qT{rep}_{lt}",
                                      tag=f"qT{lt}")
                    nc.scalar.copy(qT_sb[:], qt_ps[:])
                    qT_sbs.append(qT_sb)
                    if DBG and rep == 0 and lt == 0:
                        nc.gpsimd.dma_start(dbg_qt[:], qT_sb[:])

            for lt in range(NLT):
                with ExitStack() as plt:
                    lp = plt.enter_context(tc.tile_pool(name=f"lp{lt}", bufs=1))
                    qT_sb = qT_sbs[lt]

                    # ---------------- P2: qkT[m,(l,h)] + bias row ----------
                    qkT_sb = lp.tile([128, NC_DM * 2048], BF,
                                     name=f"qkT{rep}_{lt}", tag="qkT")
                    qkb_sb = lp.tile([1, 2048], BF, name=f"qkb{rep}_{lt}", tag="qkb")
                    with tc.tile_pool(name="qk_ps", bufs=1, space="PSUM") as qkp:
                        for c in range(NC_DM):
                            qk_ps = qkp.tile([128, 2048], F32,
                                             name=f"qkps{rep}_{lt}_{c}", tag="qkps")
                            for h in range(H):
                                nc.tensor.matmul(
                                    qk_ps[:, h * 128:(h + 1) * 128],
                                    wkz_sb[:, (c * H + h) * 128:(c * H + h + 1) * 128],
                                    qT_sb[:, (h // 2) * 128:(h // 2 + 1) * 128],
                                    start=True, stop=True, skip_group_check=True)
                            nc.scalar.copy(
                                qkT_sb[:, c * 2048:(c + 1) * 2048]
                                .rearrange("p (l h) -> p h l", h=H),
                                qk_ps.rearrange("p (h l) -> p h l", h=H))
                            if DBG and rep == 0 and lt == 0 and c == 0:
                                nc.gpsimd.dma_start(
                                    dbg_qk[:], qkT_sb[:, 0:2048])
                        for hg in range(4):
                            qkb_ps = qkp.tile([128, 512], F32,
                                              name=f"qkbp{rep}_{lt}_{hg}", tag="qkbp")
                            for hh in range(4):
                                h = hg * 4 + hh
                                nc.tensor.matmul(
                                    qkb_ps[0:1, hh * 128:(hh + 1) * 128],
                                    bkz_sb[:, h:h + 1],
                                    qT_sb[:, (h // 2) * 128:(h // 2 + 1) * 128],
                                    start=True, stop=True, skip_group_check=True)
                            nc.scalar.copy(
                                qkb_sb[0:1, :].rearrange("p (l h) -> p h l", h=H)
                                [:, hg * 4:(hg + 1) * 4, :],
                                qkb_ps[0:1, :].rearrange("p (h l) -> p h l", l=128))

                    # ---------------- P3: scores -> exp -> den -------------
                    pmat_sb = lp.tile([128, TPB * 64], BF,
                                      name=f"pmat{rep}_{lt}", tag="pmat")
                    mkf_sb = lp.tile([1, TPB * 128], BF, name=f"mkf{rep}_{lt}",
                                     tag="mkf")
                    nc.sync.dma_start(
                        mkf_sb[:], mkf_in[0:1, lt * TPB * 128:(lt + 1) * TPB * 128])
                    den_stack = ExitStack()
                    den_psp = den_stack.enter_context(
                        tc.tile_pool(name="den_ps", bufs=1, space="PSUM"))
                    den_ps = den_psp.tile([128, 512], F32, name=f"den{rep}_{lt}",
                                          tag="den")
                    zmm(den_ps[:, 0:512], 512)
                    with ExitStack() as p3:
                        scp = p3.enter_context(
                            tc.tile_pool(name="sc_ps", bufs=3, space="PSUM"))
                        xt_pool = p3.enter_context(tc.tile_pool(name="xt_pool", bufs=4))
                        for g in range(4):
                            sc_ps = scp.tile([128, 512], F32,
                                             name=f"sc{rep}_{lt}_{g}", tag="sc")
                            zmm(sc_ps[:, 0:512], 512)
                            for tt in range(8):
                                tl = g * 8 + tt
                                t = lt * TPB + tl
                                xt_sb = xt_pool.tile([128, CH * 128], BF,
                                                     name=f"xt{rep}_{t}", tag="xt")
                                nc.sync.dma_start(xt_sb[:], xt_in[t])
                                o = sc_ps[:, tt * 64:(tt + 1) * 64]
                                for c in range(NC_DM):
                                    nc.tensor.matmul(
                                        o, xt_sb[:, c * 128:(c + 1) * 128],
                                        qkT_sb[:, c * 2048 + tl * 64:
                                               c * 2048 + tl * 64 + 64],
                                        start=False, stop=False,
                                        skip_group_check=True)
                                nc.tensor.matmul(
                                    o, xt_sb[0:1, NC_DM * 128:NC_DM * 128 + 128],
                                    qkb_sb[0:1, tl * 64:tl * 64 + 64],
                                    start=False, stop=False, skip_group_check=True)
                                nc.tensor.matmul(
                                    o, a4_sb[:], bneg_sb[:],
                                    start=False, stop=False, skip_group_check=True)
                                nc.tensor.matmul(
                                    o, mkf_sb[0:1, tl * 128:(tl + 1) * 128],
                                    ones_sb[0:1, 0:64],
                                    start=False, stop=(tt == 7),
                                    skip_group_check=True)
                            nc.scalar.activation(
                                pmat_sb[:, g * 512:(g + 1) * 512], sc_ps[:], AF.Exp)
                            for tt in range(8):
                                tl = g * 8 + tt
                                nc.tensor.matmul(
                                    den_ps[0:32, 0:64],
                                    ucol_sb[:, tl * 32:(tl + 1) * 32],
                                    pmat_sb[:, tl * 64:(tl + 1) * 64],
                                    start=False, stop=(tl == TPB - 1),
                                    skip_group_check=True)
                    rd_sb = lp.tile([32, 64], F32, name=f"rd{rep}_{lt}", tag="rd")
                    nc.vector.reciprocal(rd_sb[:], den_ps[0:32, 0:64])
                    den_stack.close()
                    rdr_sb = lp.tile([128, 16], F32, name=f"rdr{rep}_{lt}", tag="rdr")
                    with tc.tile_pool(name="rdr_ps", bufs=1, space="PSUM") as rpp:
                        rdr_ps = rpp.tile([128, 16], F32, name=f"rdp{rep}_{lt}",
                                          tag="rdp")
                        for lq in range(4):
                            nc.tensor.matmul(
                                rdr_ps[:, 0:16],
                                selp_sb[:, lq * 128:(lq + 1) * 128],
                                rd_sb[:, lq * 16:(lq + 1) * 16],
                                start=(lq == 0), stop=(lq == 3),
                                skip_group_check=True)
                        nc.vector.tensor_copy(rdr_sb[:], rdr_ps[:, 0:16])
                    if DBG and rep == 0 and lt == 0:
                        nc.gpsimd.dma_start(dbg_pm[:], pmat_sb[:])
                        nc.gpsimd.dma_start(dbg_rd[:], rdr_sb[:])

                    # -------- P5+P6: tmixT (half l-tile) + ctx accumulate ---
                    ctx_psp = plt.enter_context(
                        tc.tile_pool(name="ctx_ps", bufs=1, space="PSUM"))
                    ctx_ps = ctx_psp.tile([128, 1024], F32, name=f"ctx{rep}_{lt}",
                                          tag="ctx")
                    zmm(ctx_ps[:, 0:512], 512)
                    zmm(ctx_ps[:, 512:1024], 512)
                    HT = TPB // 2  # 16 tiles per half
                    for half in range(2):
                        # tmx layout: col = c*1024 + h*64 + t*4 + l  -> ctx lhsT
                        # slices are contiguous [128, 64] (t,l)-blocks
                        tmx_sb = lp.tile([128, HT * 512], BF,
                                         name=f"tmx{rep}_{lt}_{half}", tag="tmx")
                        tmx_v = tmx_sb.rearrange("p (c h t l) -> p c l h t",
                                                 c=NC_DM, h=H, t=HT, l=4)
                        with ExitStack() as p5:
                            tmp_psp = p5.enter_context(
                                tc.tile_pool(name="tm_ps", bufs=2, space="PSUM"))
                            tg_pool = p5.enter_context(
                                tc.tile_pool(name="tg_pool", bufs=4))
                            for q2 in range(8):
                                tm_ps = tmp_psp.tile([128, 1024], F32,
                                                     name=f"tm{rep}_{lt}_{half}_{q2}",
                                                     tag="tm")
                                for t2 in range(2):
                                    tin = q2 * 2 + t2
                                    tl = half * HT + tin
                                    t = lt * TPB + tl
                                    tg_sb = tg_pool.tile([128, DM], BF,
                                                         name=f"tg{rep}_{t}", tag="tg")
                                    nc.sync.dma_start(tg_sb[:], tgt_in[t])
                                    for c in range(NC_DM):
                                        nc.tensor.matmul(
                                            tm_ps[:, t2 * 512 + c * 64:
                                                  t2 * 512 + (c + 1) * 64],
                                            tg_sb[:, c * 128:(c + 1) * 128],
                                            pmat_sb[:, tl * 64:(tl + 1) * 64],
                                            start=True, stop=True,
                                            skip_group_check=True)
                                    nc.scalar.copy(
                                        tmx_v[:, :, :, :, tin],
                                        tm_ps[:, t2 * 512:(t2 + 1) * 512]
                                        .rearrange("p (c l h) -> p c l h",
                                                   c=NC_DM, l=4, h=H))

                        for h in range(H):
                            for c in range(NC_DM):
                                nc.tensor.matmul(
                                    ctx_ps[64 * half:64 * (half + 1),
                                           h * 64:(h + 1) * 64],
                                    tmx_sb[:, c * 1024 + h * 64:
                                           c * 1024 + h * 64 + 64],
                                    wv_sb[:, c * 1024 + h * 64:
                                          c * 1024 + (h + 1) * 64],
                                    start=False, stop=(c == NC_DM - 1),
                                    skip_group_check=True)

                    # ---------------- P7: x = src' + ctx/den; LN1 ----------
                    x1_bf = lp.tile([128, 1024], BF, name=f"x1{rep}_{lt}", tag="x1")
                    for h in range(H):
                        nc.vector.tensor_scalar_mul(
                            x1_bf[:, h * 64:(h + 1) * 64],
                            ctx_ps[:, h * 64:(h + 1) * 64], rdr_sb[:, h:h + 1])
                    srcf_sb = lp.tile([128, 1024], BF, name=f"sf{rep}_{lt}", tag="sf")
                    nc.sync.dma_start(srcf_sb[:],
                                      srcf_in[lt * 128:(lt + 1) * 128, :])
                    x_bf = lp.tile([128, 1024], BF, name=f"x{rep}_{lt}", tag="x")
                    nc.vector.tensor_tensor(x_bf[:], x1_bf[:], srcf_sb[:], AL.add)
                    if DBG and rep == 0 and lt == 0:
                        nc.gpsimd.dma_start(dbg_x1[:], x1_bf[:])

                    def ln_stats(xin, tagp):
                        w1 = lp.tile([128, 1024], BF, name=f"w1{tagp}", tag=f"wa")
                        rsum = lp.tile([128, 1], F32, name=f"rs{tagp}", tag="rs")
                        nc.scalar.activation(w1[:], xin[:], AF.Copy, accum_out=rsum[:])
                        w2 = lp.tile([128, 1024], F32, name=f"w2{tagp}", tag=f"wb")
                        ssq = lp.tile([128, 1], F32, name=f"sq{tagp}", tag="sq")
                        nc.scalar.activation(w2[:], xin[:], AF.Square,
                                             accum_out=ssq[:])
                        mean = lp.tile([128, 1], F32, name=f"mn{tagp}", tag="mn")
                        nc.vector.tensor_scalar_mul(mean[:], rsum[:], 1.0 / DM)
                        nmean = lp.tile([128, 1], F32, name=f"nm{tagp}", tag="nm")
                        nc.vector.tensor_scalar_mul(nmean[:], rsum[:], -1.0 / DM)
                        exx = lp.tile([128, 1], F32, name=f"ex{tagp}", tag="ex")
                        nc.vector.tensor_scalar_mul(exx[:], ssq[:], 1.0 / DM)
                        m2 = lp.tile([128, 1], F32, name=f"m2{tagp}", tag="m2")
                        nc.vector.tensor_tensor(m2[:], mean[:], mean[:], AL.mult)
                        var = lp.tile([128, 1], F32, name=f"vr{tagp}", tag="vr")
                        nc.vector.tensor_tensor(var[:], exx[:], m2[:], AL.subtract)
                        std = lp.tile([128, 1], F32, name=f"sd{tagp}", tag="sd")
                        nc.scalar.activation(std[:], var[:], AF.Sqrt, bias=eps_sb[:])
                        rstd = lp.tile([128, 1], F32, name=f"rv{tagp}", tag="rv")
                        nc.vector.reciprocal(rstd[:], std[:])
                        return nmean, rstd

                    nmean, rstd = ln_stats(x_bf, f"a{rep}_{lt}")
                    xn_bf = xres.tile([128, 1024], BF, name=f"xn{rep}_{lt}",
                                      tag=f"xn{lt}")
                    xn_tiles.append(xn_bf)
                    nc.vector.tensor_scalar(
                        out=xn_bf[:], in0=x_bf[:], scalar1=nmean[:],
                        scalar2=rstd[:], op0=AL.add, op1=AL.mult)
                    if DBG and rep == 0 and lt == 0:
                        nc.gpsimd.dma_start(dbg_xn[:], xn_bf[:])

                    # ---------------- P8a: transpose xn --------------------
                    tp_psp = plt.enter_context(
                        tc.tile_pool(name="tp_ps", bufs=1, space="PSUM"))
                    tp_ps = tp_psp.tile([128, 1024], BF, name=f"tp{rep}_{lt}",
                                        tag="tp")
                    for c in range(NC_DM):
                        nc.tensor.transpose(tp_ps[:, c * 128:(c + 1) * 128],
                                            xn_bf[:, c * 128:(c + 1) * 128],
                                            ident[:])
                    nc.scalar.copy(
                        xT_sb.rearrange("p (c l) -> p c l", l=LC)
                        [:, :, lt * 128:(lt + 1) * 128],
                        tp_ps.rearrange("p (c l) -> p c l", l=128))

            # ---------------- P8: FFN (both l-tiles) -----------------------
            with ExitStack() as p8:
                f1_psp = p8.enter_context(
                    tc.tile_pool(name="f1_ps", bufs=2, space="PSUM"))
                o_psp = p8.enter_context(
                    tc.tile_pool(name="o_ps", bufs=1, space="PSUM"))
                w1_pool = p8.enter_context(tc.tile_pool(name="w1_pool", bufs=3))
                w2_pool = p8.enter_context(tc.tile_pool(name="w2_pool", bufs=3))
                ff1_sb = xtp.tile([128, NFF * LC], BF, name=f"ff1_{rep}", tag="ff1")
                for fq in range(8):
                    f1_ps = f1_psp.tile([128, 1024], F32, name=f"f1{rep}_{fq}",
                                        tag="f1")
                    zmm(f1_ps[:, 0:512], 512)
                    zmm(f1_ps[:, 512:1024], 512)
                    for f4 in range(4):
                        f = fq * 4 + f4
                        w1_sb = w1_pool.tile([128, NC_DM * 128], BF,
                                             name=f"w1s{rep}_{f}", tag="w1s")
                        nc.sync.dma_start(w1_sb[:], w1_in[f])
                        o = f1_ps[:, f4 * 256:(f4 + 1) * 256]
                        for c in range(NC_DM):
                            nc.tensor.matmul(
                                o, w1_sb[:, c * 128:(c + 1) * 128],
                                xT_sb[:, c * LC:(c + 1) * LC],
                                start=False, stop=False, skip_group_check=True)
                        nc.tensor.matmul(
                            o, b1r_sb[0:1, f * 128:(f + 1) * 128],
                            ones_sb[0:1, 0:256],
                            start=False, stop=True, skip_group_check=True)
                    nc.scalar.activation(ff1_sb[:, fq * 1024:(fq + 1) * 1024],
                                         f1_ps[:], AF.Gelu)
                    if DBG and rep == 0 and fq == 0:
                        nc.gpsimd.dma_start(dbg_f1[:], ff1_sb[:, 0:1024])
                        nc.gpsimd.dma_start(dbg_xt[:], xT_sb[:, 0:2048])

                o_ps = [o_psp.tile([128, 1024], F32, name=f"o{rep}_{i}",
                                   tag=f"o{i}") for i in range(NLT)]
                for i in range(NLT):
                    zmm(o_ps[i][:, 0:512], 512)
                    zmm(o_ps[i][:, 512:1024], 512)
                    for hh in range(2):
                        nc.tensor.matmul(
                            o_ps[i][:, hh * 512:(hh + 1) * 512],
                            ones_sb[0:1, 0:128],
                            b2r_sb[0:1, hh * 512:(hh + 1) * 512],
                            start=False, stop=False, skip_group_check=True)
                for f in range(NFF):
                    w2_sb = w2_pool.tile([128, DM], BF, name=f"w2s{rep}_{f}",
                                         tag="w2s")
                    nc.sync.dma_start(w2_sb[:], w2_in[f])
                    for i in range(NLT):
                        for hh in range(2):
                            nc.tensor.matmul(
                                o_ps[i][:, hh * 512:(hh + 1) * 512],
                                ff1_sb[:, f * LC + i * 128: f * LC + (i + 1) * 128],
                                w2_sb[:, hh * 512:(hh + 1) * 512],
                                start=False, stop=(f == NFF - 1),
                                skip_group_check=True)

                # ---------------- P9: residual + LN2 + out -----------------
                lp2 = p8.enter_context(tc.tile_pool(name="lp2", bufs=2))
                eps2 = eps_sb
                for i in range(NLT):
                    ff_bf = lp2.tile([128, 1024], BF, name=f"ffb{rep}_{i}", tag="ffb")
                    nc.scalar.copy(ff_bf[:], o_ps[i][:])
                    t1 = lp2.tile([128, 1024], BF, name=f"t1{rep}_{i}", tag="t1")
                    nc.vector.tensor_tensor(t1[:], xn_tiles[i][:], g1r_sb[:], AL.mult)
                    r2 = lp2.tile([128, 1024], BF, name=f"r2{rep}_{i}", tag="r2")
                    nc.vector.tensor_tensor(r2[:], t1[:], ff_bf[:], AL.add)

                    wv1 = lp2.tile([128, 1024], BF, name=f"wv1{rep}_{i}", tag="wv1")
                    rsum = lp2.tile([128, 1], F32, name=f"rs2{rep}_{i}", tag="rs2")
                    nc.scalar.activation(wv1[:], r2[:], AF.Copy, accum_out=rsum[:])
                    wv2 = lp2.tile([128, 1024], F32, name=f"wv2{rep}_{i}", tag="wv2")
                    ssq = lp2.tile([128, 1], F32, name=f"sq2{rep}_{i}", tag="sq2")
                    nc.scalar.activation(wv2[:], r2[:], AF.Square, accum_out=ssq[:])
                    mean = lp2.tile([128, 1], F32, name=f"mn2{rep}_{i}", tag="mn2")
                    nc.vector.tensor_scalar_mul(mean[:], rsum[:], 1.0 / DM)
                    nmean = lp2.tile([128, 1], F32, name=f"nm2{rep}_{i}", tag="nm2")
                    nc.vector.tensor_scalar_mul(nmean[:], rsum[:], -1.0 / DM)
                    exx = lp2.tile([128, 1], F32, name=f"ex2{rep}_{i}", tag="ex2")
                    nc.vector.tensor_scalar_mul(exx[:], ssq[:], 1.0 / DM)
                    m2 = lp2.tile([128, 1], F32, name=f"m22{rep}_{i}", tag="m22")
                    nc.vector.tensor_tensor(m2[:], mean[:], mean[:], AL.mult)
                    var = lp2.tile([128, 1], F32, name=f"vr2{rep}_{i}", tag="vr2")
                    nc.vector.tensor_tensor(var[:], exx[:], m2[:], AL.subtract)
                    std = lp2.tile([128, 1], F32, name=f"sd2{rep}_{i}", tag="sd2")
                    nc.scalar.activation(std[:], var[:], AF.Sqrt, bias=eps2[:])
                    rstd = lp2.tile([128, 1], F32, name=f"rv2{rep}_{i}", tag="rv2")
                    nc.vector.reciprocal(rstd[:], std[:])
                    xn2 = lp2.tile([128, 1024], BF, name=f"xn2{rep}_{i}", tag="xn2")
                    nc.vector.tensor_scalar(
                        out=xn2[:], in0=r2[:], scalar1=nmean[:], scalar2=rstd[:],
                        op0=AL.add, op1=AL.mult)
                    t3 = lp2.tile([128, 1024], BF, name=f"t3{rep}_{i}", tag="t3")
                    nc.vector.tensor_tensor(t3[:], xn2[:], g2r_sb[:], AL.mult)
                    y_bf = lp2.tile([128, 1024], BF, name=f"y{rep}_{i}", tag="y")
                    nc.vector.tensor_tensor(y_bf[:], t3[:], be2_sb[:], AL.add)
                    nc.gpsimd.dma_start(out[i * 128:(i + 1) * 128, :], y_bf[:])

    nc.compile()
    return nc


def _prep_core(src_c, tgt_c, mask_c, W):
    bf = ml_dtypes.bfloat16
    X = np.ascontiguousarray(tgt_c.reshape(LC * D, DM)).astype(np.float32)

    xt = np.zeros((NT, 128, CH, 128), dtype=np.float32)
    xt[:, :, :NC_DM, :] = X.reshape(NT, 128, NC_DM, 128).transpose(0, 3, 2, 1)
    xt[:, 0, NC_DM, :] = 1.0

    tgt = X.reshape(NT, 128, DM)

    st = np.zeros((128, CH, LC), dtype=np.float32)
    st[:, :NC_DM, :] = src_c.reshape(LC, NC_DM, 128).transpose(2, 1, 0)
    st[0, NC_DM, :] = 1.0

    Wp = np.zeros((CH * 128, DM), dtype=np.float32)
    Wp[:DM] = W["Wq"] / SCALE
    Wp[DM] = W["bq"] / SCALE
    wq = Wp.reshape(CH, 128, DM).transpose(1, 0, 2).reshape(128, CH * 1024)

    # wkz[p, (c*H+h)*128 + j] = Wk[c*128+j, h*64 + (p - 64*(h%2))] if p//64==h%2
    Wk = W["Wk"].reshape(NC_DM, 128, H, DH)  # [c, j, h, dh]
    wkz = np.zeros((128, NC_DM, H, 128), dtype=np.float32)
    for e in range(2):
        # heads with h%2==e sit at partitions 64e..64e+64
        wkz[64 * e:64 * (e + 1), :, e::2, :] = Wk[:, :, e::2, :].transpose(3, 0, 2, 1)
    wkz = wkz.reshape(128, NC_DM * H * 128)

    bk = W["bk"].reshape(H, DH)
    bkz = np.zeros((128, H), dtype=np.float32)
    for h in range(H):
        e = h % 2
        bkz[64 * e:64 * (e + 1), h] = bk[h]

    wv = np.ascontiguousarray(
        W["Wv"].reshape(NC_DM, 128, DM).transpose(1, 0, 2).reshape(128, NC_DM * 1024))

    mkf = np.zeros((1, NT * 128), dtype=np.float32)
    m = mask_c.reshape(NT, 4, D)  # [t, r//32, d]
    mkf[0] = np.repeat(m, 1, axis=1).transpose(0, 1, 2).reshape(NT, 128).reshape(-1)

    srcf = src_c + W["bv"][None, :]

    W1p = W["g1"][:, None] * W["W1"]  # fold LN1 gain
    w1p = np.ascontiguousarray(
        W1p.reshape(NC_DM, 128, NFF, 128).transpose(2, 1, 0, 3)
        .reshape(NFF, 128, NC_DM * 128))
    b1r = (W["beta1"] @ W["W1"] + W["b1"]).reshape(1, NFF * 128)
    w2p = np.ascontiguousarray(W["W2"].reshape(NFF, 128, DM))
    b2r = (W["b2"] + W["beta1"]).reshape(1, DM)

    return {
        "xt": xt.reshape(NT, 128, CH * 128).astype(bf),
        "tgt": tgt.astype(bf),
        "st": st.reshape(128, CH * LC).astype(bf),
        "wq": wq.astype(bf),
        "wkz": wkz.astype(bf),
        "bkz": bkz.astype(bf),
        "wv": wv.astype(bf),
        "mkf": mkf.astype(bf),
        "srcf": srcf.astype(bf),
        "w1p": w1p.astype(bf),
        "b1r": b1r.astype(bf),
        "w2p": w2p.astype(bf),
        "b2r": b2r.astype(bf),
        "g1r": np.tile(W["g1"], (128, 1)).astype(bf),
        "g2r": np.tile(W["g2"], (128, 1)).astype(bf),
        "be2r": np.tile(W["beta2"], (128, 1)).astype(bf),
    }


def make_in_maps(**inputs):
    inp = {k: np.asarray(v) for k, v in inputs.items()}
    W = {k: inp[k] for k in ("Wq", "bq", "Wk", "bk", "Wv", "bv", "W1", "b1",
                             "W2", "b2", "g1", "beta1", "g2", "beta2")}
    in_maps = []
    for c in range(NCORES):
        sl = slice(c * LC, (c + 1) * LC)
        in_maps.append(
            _prep_core(inp["src"][sl], inp["target"][sl], inp["attn_mask"][sl], W))
    return in_maps


def get_nc(repeat=1):
    key = ("nc", repeat)
    if key not in _CACHE:
        _CACHE[key] = _build_nc(repeat)
    return _CACHE[key]


def kernel(**inputs) -> np.ndarray:
    nc = get_nc()
    in_maps = make_in_maps(**inputs)
    res = run_bass_kernel_spmd(nc, in_maps, core_ids=list(range(NCORES)))
    return np.concatenate([res.results[c]["out"] for c in range(NCORES)], axis=0)


if __name__ == "__main__":
    import reference

    inputs = {k: np.asarray(v) for k, v in reference.setup_inputs().items()}
    got = kernel(**inputs)
    exp = np.asarray(reference.reference(**inputs))
    err = np.abs(got - exp).max() / np.abs(exp).max()
    print("Relative error:", err)


# revision 3
# speedup vs baseline: 1.0793x; 1.0793x over previous
"""Trainium2 Bass kernel v2 for nn_ContextEncoderLayer.

Key redesign vs v1 (which was DVE-bound at ~38us per big f32 DVE op on this HW):
  - scores via PE "query pullback": qk[m,(l,h)] = sum_dh Wk[m,(h,dh)] q[l,(h,dh)],
    then scores[row,(l,h)] = sum_m target[row,m] qk[m,(l,h)] -- all matmuls.
    Block-diagonal validity mask, attn mask and k-bias folded into the PE
    accumulation as tiny extra matmuls, so exp() batches 8 tiles per ACT op.
  - V path via associativity: ctx = (P . target) @ Wv.  tmixT[m,(l,h)] comes out
    of PE transpose-ready; evacuated in [128,2048] ACT ops.
  - All PSUM evacuation on ACT (scalar) engine; DVE only does bf16 2x-mode ops
    and tiny [128,1] scalars (f32 DVE ops >=[128,512] are pathologically slow).
  - LN affine g1/beta1 folded into W1/b1 host-side; bv folded into src; beta1+b2
    folded into the ff2 bias row.
"""

import sys

sys.path.insert(0, "/opt/trn_rl_repo")

from contextlib import ExitStack

import numpy as np
import ml_dtypes

import concourse.bacc as bacc
import concourse.tile as tile
from concourse import mybir
from concourse.bass_utils import run_bass_kernel_spmd
from concourse.masks import make_identity

import os
DBG = bool(os.environ.get("K2DBG"))
L, D, DM, H, FF = 2048, 32, 1024, 16, 4096
DH = DM // H  # 64
SCALE = float(np.sqrt(DH))  # 8.0
NCORES = 8
LC = L // NCORES  # 256 positions per core
NT = LC * D // 128  # 64 row tiles per core (128 rows = 4 pos x 32 cands)
NLT = LC // 128  # 2 l-tiles
NC_DM = DM // 128  # 8
CH = NC_DM + 1  # 9 (ones row for bias folds)
NFF = FF // 128  # 32
TPB = NT // NLT  # 32 tiles per l-tile
BF = mybir.dt.bfloat16
F32 = mybir.dt.float32
NEG = -1.0e30

_CACHE = {}


def _consts():
    a4 = np.zeros((4, 128), np.float32)
    for r in range(128):
        a4[r // 32, r] = 1.0
    bneg = np.full((4, 64), NEG, np.float32)
    for i in range(4):
        for h in range(H):
            bneg[i, i * 16 + h] = 0.0
    ucol = np.zeros((128, 32 * 32), np.float32)
    for j in range(32):
        ucol[:, j * 32 + j] = 1.0
    selp = np.zeros((32, 4 * 128), np.float32)
    for t in range(32):
        for lp in range(4):
            selp[t, lp * 128 + 4 * t + lp] = 1.0
    ones = np.ones((1, 256), np.float32)
    zrow = np.zeros((1, 512), np.float32)
    bf = ml_dtypes.bfloat16
    return (a4.astype(bf), bneg.astype(bf), ucol.astype(bf), ones.astype(bf),
            zrow.astype(bf), selp)


def _build_nc(repeat=1):
    nc = bacc.Bacc("TRN2", target_bir_lowering=False, debug=False, num_devices=NCORES)

    xt_in = nc.dram_tensor("xt", [NT, 128, CH * 128], BF, kind="ExternalInput")
    tgt_in = nc.dram_tensor("tgt", [NT, 128, DM], BF, kind="ExternalInput")
    st_in = nc.dram_tensor("st", [128, CH * LC], BF, kind="ExternalInput")
    wq_in = nc.dram_tensor("wq", [128, CH * 1024], BF, kind="ExternalInput")
    wkz_in = nc.dram_tensor("wkz", [128, NC_DM * H * 128], BF, kind="ExternalInput")
    bkz_in = nc.dram_tensor("bkz", [128, H], BF, kind="ExternalInput")
    wv_in = nc.dram_tensor("wv", [128, NC_DM * 1024], BF, kind="ExternalInput")
    mkf_in = nc.dram_tensor("mkf", [1, NT * 128], BF, kind="ExternalInput")
    srcf_in = nc.dram_tensor("srcf", [LC, DM], BF, kind="ExternalInput")
    w1_in = nc.dram_tensor("w1p", [NFF, 128, NC_DM * 128], BF, kind="ExternalInput")
    b1r_in = nc.dram_tensor("b1r", [1, NFF * 128], BF, kind="ExternalInput")
    w2_in = nc.dram_tensor("w2p", [NFF, 128, DM], BF, kind="ExternalInput")
    b2r_in = nc.dram_tensor("b2r", [1, DM], BF, kind="ExternalInput")
    g1r_in = nc.dram_tensor("g1r", [128, DM], BF, kind="ExternalInput")
    g2r_in = nc.dram_tensor("g2r", [128, DM], BF, kind="ExternalInput")
    be2_in = nc.dram_tensor("be2r", [128, DM], BF, kind="ExternalInput")
    out = nc.dram_tensor("out", [LC, DM], F32, kind="ExternalOutput")
    if DBG:
        dbg_qt = nc.dram_tensor("dbg_qt", [128, 1024], F32, kind="ExternalOutput")
        dbg_qk = nc.dram_tensor("dbg_qk", [128, 2048], F32, kind="ExternalOutput")
        dbg_pm = nc.dram_tensor("dbg_pm", [128, 2048], F32, kind="ExternalOutput")
        dbg_rd = nc.dram_tensor("dbg_rd", [128, 16], F32, kind="ExternalOutput")
        dbg_x1 = nc.dram_tensor("dbg_x1", [128, 1024], F32, kind="ExternalOutput")
        dbg_xn = nc.dram_tensor("dbg_xn", [128, 1024], F32, kind="ExternalOutput")
        dbg_xt = nc.dram_tensor("dbg_xt", [128, 2048], F32, kind="ExternalOutput")
        dbg_f1 = nc.dram_tensor("dbg_f1", [128, 1024], F32, kind="ExternalOutput")

    a4_np, bneg_np, ucol_np, ones_np, zrow_np, selp_np = _consts()
    selp_c = nc.inline_tensor(selp_np, name="selpc")
    a4_c = nc.inline_tensor(a4_np, name="a4c")
    bneg_c = nc.inline_tensor(bneg_np, name="bnegc")
    ucol_c = nc.inline_tensor(ucol_np, name="ucolc")
    ones_c = nc.inline_tensor(ones_np, name="onesc")
    zrow_c = nc.inline_tensor(zrow_np, name="zrowc")

    AL = mybir.AluOpType
    AF = mybir.ActivationFunctionType

    with tile.TileContext(nc) as tc, ExitStack() as top:
        consts = top.enter_context(tc.tile_pool(name="consts", bufs=1))

        def cload(name, src, shape, dt=BF):
            t = consts.tile(shape, dt, name=name)
            nc.sync.dma_start(t[:], src[:])
            return t

        wkz_sb = cload("wkz_sb", wkz_in, [128, NC_DM * H * 128])
        bkz_sb = cload("bkz_sb", bkz_in, [128, H])
        wv_sb = cload("wv_sb", wv_in, [128, NC_DM * 1024])
        b1r_sb = cload("b1r_sb", b1r_in, [1, NFF * 128])
        b2r_sb = cload("b2r_sb", b2r_in, [1, DM])
        g1r_sb = cload("g1r_sb", g1r_in, [128, DM])
        g2r_sb = cload("g2r_sb", g2r_in, [128, DM])
        be2_sb = cload("be2_sb", be2_in, [128, DM])
        a4_sb = consts.tile([4, 128], BF, name="a4_sb")
        nc.sync.dma_start(a4_sb[:], a4_c[:])
        bneg_sb = consts.tile([4, 64], BF, name="bneg_sb")
        nc.sync.dma_start(bneg_sb[:], bneg_c[:])
        ucol_sb = consts.tile([128, 32 * 32], BF, name="ucol_sb")
        nc.sync.dma_start(ucol_sb[:], ucol_c[:])
        ones_sb = consts.tile([1, 256], BF, name="ones_sb")
        nc.sync.dma_start(ones_sb[:], ones_c[:])
        zrow_sb = consts.tile([1, 512], BF, name="zrow_sb")
        nc.sync.dma_start(zrow_sb[:], zrow_c[:])
        selp_sb = consts.tile([32, 4 * 128], F32, name="selp_sb")
        nc.sync.dma_start(selp_sb[:], selp_c[:])
        ident = consts.tile([128, 128], BF, name="ident")
        make_identity(nc, ident[:])
        eps_sb = consts.tile([128, 1], F32, name="eps_sb")
        nc.vector.memset(eps_sb[:], 1e-5)

        def zmm(ps_slice, n):
            # write zeros to [128, n] psum region (sets has_written -> later
            # matmuls with start=False accumulate safely in any order)
            nc.tensor.matmul(ps_slice, ones_sb[0:1, 0:128], zrow_sb[0:1, 0:n],
                             start=True, stop=False, skip_group_check=True)

        xres = top.enter_context(tc.tile_pool(name="xres", bufs=1))
        xtp = top.enter_context(tc.tile_pool(name="xtp", bufs=1))

        for rep in range(repeat):
            xn_tiles = []
            xT_sb = xtp.tile([128, NC_DM * LC], BF, name=f"xT{rep}", tag="xT")

            mkf_all = xtp.tile([1, NT * 128], BF, name=f"mkfa{rep}", tag="mkfa")
            nc.sync.dma_start(mkf_all[:], mkf_in[:])

            # ---------------- P1: qT = Wq^T srcT  (both l-tiles) -----------
            qT_sbs = []
            with tc.tile_pool(name="p1pool", bufs=1) as p1p, \
                 tc.tile_pool(name="qt_ps", bufs=1, space="PSUM") as qt_psp:
                st_sb = p1p.tile([128, CH * LC], BF, name=f"st{rep}", tag="st")
                nc.sync.dma_start(st_sb[:], st_in[:])
                wq_sb = p1p.tile([128, CH * 1024], BF, name=f"wqs{rep}", tag="wqs")
                nc.sync.dma_start(wq_sb[:], wq_in[:])
                for lt in range(NLT):
                    qt_ps = qt_psp.tile([128, 1024], F32, name=f"qtps{rep}_{lt}",
                                        tag=f"qtps{lt}")
                    zmm(qt_ps[:, 0:512], 512)
                    zmm(qt_ps[:, 512:1024], 512)
                    for o in range(8):
                        for c in range(CH):
                            pr = slice(0, 128) if c < NC_DM else slice(0, 1)
                            nc.tensor.matmul(
                                qt_ps[:, o * 128:(o + 1) * 128],
                                wq_sb[pr, c * 1024 + o * 128: c * 1024 + (o + 1) * 128],
                                st_sb[pr, c * LC + lt * 128: c * LC + (lt + 1) * 128],
                                start=False, stop=(c == CH - 1),
                                skip_group_check=True)
                    qT_sb = xres.tile([128, 1024], BF, name=f"qT{rep}_{lt}",
                                      tag=f"qT{lt}")
                    nc.scalar.copy(qT_sb[:], qt_ps[:])
                    qT_sbs.append(qT_sb)
                    if DBG and rep == 0 and lt == 0:
                        nc.gpsimd.dma_start(dbg_qt[:], qT_sb[:])

            for lt in range(NLT):
                with ExitStack() as plt:
                    lp = plt.enter_context(tc.tile_pool(name=f"lp{lt}", bufs=1))
                    qT_sb = qT_sbs[lt]

                    # ---------------- P2: qkT[m,(l,h)] + bias row ----------
                    qkT_sb = lp.tile([128, NC_DM * 2048], BF,
                                     name=f"qkT{rep}_{lt}", tag="qkT")
                    qkb_sb = lp.tile([1, 2048], BF, name=f"qkb{rep}_{lt}", tag="qkb")
                    with tc.tile_pool(name="qk_ps", bufs=1, space="PSUM") as qkp:
                        for c in range(NC_DM):
                            qk_ps = qkp.tile([128, 2048], F32,
                                             name=f"qkps{rep}_{lt}_{c}", tag="qkps")
                            for h in range(H):
                                nc.tensor.matmul(
                                    qk_ps[:, h * 128:(h + 1) * 128],
                                    wkz_sb[:, (c * H + h) * 128:(c * H + h + 1) * 128],
                                    qT_sb[:, (h // 2) * 128:(h // 2 + 1) * 128],
                                    start=True, stop=True, skip_group_check=True)
                            nc.scalar.copy(
                                qkT_sb[:, c * 2048:(c + 1) * 2048]
                                .rearrange("p (l h) -> p h l", h=H),
                                qk_ps.rearrange("p (h l) -> p h l", h=H))
                            if DBG and rep == 0 and lt == 0 and c == 0:
                                nc.gpsimd.dma_start(
                                    dbg_qk[:], qkT_sb[:, 0:2048])
                        for hg in range(4):
                            qkb_ps = qkp.tile([128, 512], F32,
                                              name=f"qkbp{rep}_{lt}_{hg}", tag="qkbp")
                            for hh in range(4):
                                h = hg * 4 + hh
                                nc.tensor.matmul(
                                    qkb_ps[0:1, hh * 128:(hh + 1) * 128],
                                    bkz_sb[:, h:h + 1],
                                    qT_sb[:, (h // 2) * 128:(h // 2 + 1) * 128],
                                    start=True, stop=True, skip_group_check=True)
                            nc.scalar.copy(
                                qkb_sb[0:1, :].rearrange("p (l h) -> p h l", h=H)
                                [:, hg * 4:(hg + 1) * 4, :],
                                qkb_ps[0:1, :].rearrange("p (h l) -> p h l", l=128))

                    # ---------------- P3: scores -> exp -> den -------------
                    pmat_sb = lp.tile([128, TPB * 64], BF,
                                      name=f"pmat{rep}_{lt}", tag="pmat")
                    mkf_sb = mkf_all[:, lt * TPB * 128:(lt + 1) * TPB * 128]
                    den_stack = ExitStack()
                    den_psp = den_stack.enter_context(
                        tc.tile_pool(name="den_ps", bufs=1, space="PSUM"))
                    den_ps = den_psp.tile([128, 512], F32, name=f"den{rep}_{lt}",
                                          tag="den")
                    zmm(den_ps[:, 0:512], 512)
                    with ExitStack() as p3:
                        scp = p3.enter_context(
                            tc.tile_pool(name="sc_ps", bufs=3, space="PSUM"))
                        xt_pool = p3.enter_context(tc.tile_pool(name="xt_pool", bufs=2))
                        for g in range(4):
                            sc_ps = scp.tile([128, 512], F32,
                                             name=f"sc{rep}_{lt}_{g}", tag="sc")
                            zmm(sc_ps[:, 0:512], 512)
                            xt4s = []
                            for sub in range(2):
                                t0 = lt * TPB + g * 8 + sub * 4
                                xt4 = xt_pool.tile([128, 4 * CH * 128], BF,
                                                   name=f"xt{rep}_{t0}", tag="xt")
                                nc.sync.dma_start(
                                    xt4.rearrange("p (t x) -> p t x", t=4),
                                    xt_in[t0:t0 + 4].rearrange("t p x -> p t x"))
                                xt4s.append(xt4)
                            for tt in range(8):
                                tl = g * 8 + tt
                                xt_sb = xt4s[tt // 4]
                                xo = (tt % 4) * CH * 128
                                o = sc_ps[:, tt * 64:(tt + 1) * 64]
                                for c in range(NC_DM):
                                    nc.tensor.matmul(
                                        o, xt_sb[:, xo + c * 128: xo + (c + 1) * 128],
                                        qkT_sb[:, c * 2048 + tl * 64:
                                               c * 2048 + tl * 64 + 64],
                                        start=False, stop=False,
                                        skip_group_check=True)
                                nc.tensor.matmul(
                                    o, xt_sb[0:1, xo + NC_DM * 128:
                                             xo + NC_DM * 128 + 128],
                                    qkb_sb[0:1, tl * 64:tl * 64 + 64],
                                    start=False, stop=False, skip_group_check=True)
                                nc.tensor.matmul(
                                    o, a4_sb[:], bneg_sb[:],
                                    start=False, stop=False, skip_group_check=True)
                                nc.tensor.matmul(
                                    o, mkf_sb[0:1, tl * 128:(tl + 1) * 128],
                                    ones_sb[0:1, 0:64],
                                    start=False, stop=(tt == 7),
                                    skip_group_check=True)
                            nc.scalar.activation(
                                pmat_sb[:, g * 512:(g + 1) * 512], sc_ps[:], AF.Exp)
                            for tt in range(8):
                                tl = g * 8 + tt
                                nc.tensor.matmul(
                                    den_ps[0:32, 0:64],
                                    ucol_sb[:, tl * 32:(tl + 1) * 32],
                                    pmat_sb[:, tl * 64:(tl + 1) * 64],
                                    start=False, stop=(tl == TPB - 1),
                                    skip_group_check=True)
                    rd_sb = lp.tile([32, 64], F32, name=f"rd{rep}_{lt}", tag="rd")
                    nc.vector.reciprocal(rd_sb[:], den_ps[0:32, 0:64])
                    den_stack.close()
                    rdr_sb = lp.tile([128, 16], F32, name=f"rdr{rep}_{lt}", tag="rdr")
                    with tc.tile_pool(name="rdr_ps", bufs=1, space="PSUM") as rpp:
                        rdr_ps = rpp.tile([128, 16], F32, name=f"rdp{rep}_{lt}",
                                          tag="rdp")
                        for lq in range(4):
                            nc.tensor.matmul(
                                rdr_ps[:, 0:16],
                                selp_sb[:, lq * 128:(lq + 1) * 128],
                                rd_sb[:, lq * 16:(lq + 1) * 16],
                                start=(lq == 0), stop=(lq == 3),
                                skip_group_check=True)
                        nc.vector.tensor_copy(rdr_sb[:], rdr_ps[:, 0:16])
                    if DBG and rep == 0 and lt == 0:
                        nc.gpsimd.dma_start(dbg_pm[:], pmat_sb[:])
                        nc.gpsimd.dma_start(dbg_rd[:], rdr_sb[:])

                    # -------- P5+P6: tmixT (half l-tile) + ctx accumulate ---
                    ctx_psp = plt.enter_context(
                        tc.tile_pool(name="ctx_ps", bufs=1, space="PSUM"))
                    ctx_ps = ctx_psp.tile([128, 1024], F32, name=f"ctx{rep}_{lt}",
                                          tag="ctx")
                    zmm(ctx_ps[:, 0:512], 512)
                    zmm(ctx_ps[:, 512:1024], 512)
                    HT = TPB // 2  # 16 tiles per half
                    for half in range(2):
                        # tmx layout: col = c*1024 + h*64 + t*4 + l  -> ctx lhsT
                        # slices are contiguous [128, 64] (t,l)-blocks
                        tmx_sb = lp.tile([128, HT * 512], BF,
                                         name=f"tmx{rep}_{lt}_{half}", tag="tmx")
                        tmx_v = tmx_sb.rearrange("p (c h t l) -> p c l h t",
                                                 c=NC_DM, h=H, t=HT, l=4)
                        with ExitStack() as p5:
                            tmp_psp = p5.enter_context(
                                tc.tile_pool(name="tm_ps", bufs=2, space="PSUM"))
                            tg_pool = p5.enter_context(
                                tc.tile_pool(name="tg_pool", bufs=2))
                            for q4 in range(4):
                                t0 = lt * TPB + half * HT + q4 * 4
                                tg4 = tg_pool.tile([128, 4 * DM], BF,
                                                   name=f"tg{rep}_{t0}", tag="tg")
                                nc.sync.dma_start(
                                    tg4.rearrange("p (t x) -> p t x", t=4),
                                    tgt_in[t0:t0 + 4].rearrange("t p x -> p t x"))
                                for q2 in range(2):
                                    tm_ps = tmp_psp.tile(
                                        [128, 1024], F32,
                                        name=f"tm{rep}_{lt}_{half}_{q4}_{q2}",
                                        tag="tm")
                                    for t2 in range(2):
                                        ti4 = q2 * 2 + t2
                                        tin = q4 * 4 + ti4
                                        tl = half * HT + tin
                                        to = ti4 * DM
                                        for c in range(NC_DM):
                                            nc.tensor.matmul(
                                                tm_ps[:, t2 * 512 + c * 64:
                                                      t2 * 512 + (c + 1) * 64],
                                                tg4[:, to + c * 128:
                                                    to + (c + 1) * 128],
                                                pmat_sb[:, tl * 64:(tl + 1) * 64],
                                                start=True, stop=True,
                                                skip_group_check=True)
                                        nc.scalar.copy(
                                            tmx_v[:, :, :, :, tin],
                                            tm_ps[:, t2 * 512:(t2 + 1) * 512]
                                            .rearrange("p (c l h) -> p c l h",
                                                       c=NC_DM, l=4, h=H))

                        for h in range(H):
                            for c in range(NC_DM):
                                nc.tensor.matmul(
                                    ctx_ps[64 * half:64 * (half + 1),
                                           h * 64:(h + 1) * 64],
                                    tmx_sb[:, c * 1024 + h * 64:
                                           c * 1024 + h * 64 + 64],
                                    wv_sb[:, c * 1024 + h * 64:
                                          c * 1024 + (h + 1) * 64],
                                    start=False, stop=(c == NC_DM - 1),
                                    skip_group_check=True)

                    # ---------------- P7: x = src' + ctx/den; LN1 ----------
                    x1_bf = lp.tile([128, 1024], BF, name=f"x1{rep}_{lt}", tag="x1")
                    for h in range(H):
                        nc.vector.tensor_scalar_mul(
                            x1_bf[:, h * 64:(h + 1) * 64],
                            ctx_ps[:, h * 64:(h + 1) * 64], rdr_sb[:, h:h + 1])
                    srcf_sb = lp.tile([128, 1024], BF, name=f"sf{rep}_{lt}", tag="sf")
                    nc.sync.dma_start(srcf_sb[:],
                                      srcf_in[lt * 128:(lt + 1) * 128, :])
                    x_bf = lp.tile([128, 1024], BF, name=f"x{rep}_{lt}", tag="x")
                    nc.vector.tensor_tensor(x_bf[:], x1_bf[:], srcf_sb[:], AL.add)
                    if DBG and rep == 0 and lt == 0:
                        nc.gpsimd.dma_start(dbg_x1[:], x1_bf[:])

                    def ln_stats(xin, tagp):
                        w1 = lp.tile([128, 1024], BF, name=f"w1{tagp}", tag=f"wa")
                        rsum = lp.tile([128, 1], F32, name=f"rs{tagp}", tag="rs")
                        nc.scalar.activation(w1[:], xin[:], AF.Copy, accum_out=rsum[:])
                        w2 = lp.tile([128, 1024], F32, name=f"w2{tagp}", tag=f"wb")
                        ssq = lp.tile([128, 1], F32, name=f"sq{tagp}", tag="sq")
                        nc.scalar.activation(w2[:], xin[:], AF.Square,
                                             accum_out=ssq[:])
                        mean = lp.tile([128, 1], F32, name=f"mn{tagp}", tag="mn")
                        nc.vector.tensor_scalar_mul(mean[:], rsum[:], 1.0 / DM)
                        nmean = lp.tile([128, 1], F32, name=f"nm{tagp}", tag="nm")
                        nc.vector.tensor_scalar_mul(nmean[:], rsum[:], -1.0 / DM)
                        exx = lp.tile([128, 1], F32, name=f"ex{tagp}", tag="ex")
                        nc.vector.tensor_scalar_mul(exx[:], ssq[:], 1.0 / DM)
                        m2 = lp.tile([128, 1], F32, name=f"m2{tagp}", tag="m2")
                        nc.vector.tensor_tensor(m2[:], mean[:], mean[:], AL.mult)
                        var = lp.tile([128, 1], F32, name=f"vr{tagp}", tag="vr")
                        nc.vector.tensor_tensor(var[:], exx[:], m2[:], AL.subtract)
                        std = lp.tile([128, 1], F32, name=f"sd{tagp}", tag="sd")
                        nc.scalar.activation(std[:], var[:], AF.Sqrt, bias=eps_sb[:])
                        rstd = lp.tile([128, 1], F32, name=f"rv{tagp}", tag="rv")
                        nc.vector.reciprocal(rstd[:], std[:])
                        return nmean, rstd

                    nmean, rstd = ln_stats(x_bf, f"a{rep}_{lt}")
                    xn_bf = xres.tile([128, 1024], BF, name=f"xn{rep}_{lt}",
                                      tag=f"xn{lt}")
                    xn_tiles.append(xn_bf)
                    nc.vector.tensor_scalar(
                        out=xn_bf[:], in0=x_bf[:], scalar1=nmean[:],
                        scalar2=rstd[:], op0=AL.add, op1=AL.mult)
                    if DBG and rep == 0 and lt == 0:
                        nc.gpsimd.dma_start(dbg_xn[:], xn_bf[:])

                    # ---------------- P8a: transpose xn --------------------
                    tp_psp = plt.enter_context(
                        tc.tile_pool(name="tp_ps", bufs=1, space="PSUM"))
                    tp_ps = tp_psp.tile([128, 1024], BF, name=f"tp{rep}_{lt}",
                                        tag="tp")
                    for c in range(NC_DM):
                        nc.tensor.transpose(tp_ps[:, c * 128:(c + 1) * 128],
                                            xn_bf[:, c * 128:(c + 1) * 128],
                                            ident[:])
                    nc.scalar.copy(
                        xT_sb.rearrange("p (c l) -> p c l", l=LC)
                        [:, :, lt * 128:(lt + 1) * 128],
                        tp_ps.rearrange("p (c l) -> p c l", l=128))

            # ---------------- P8: FFN (both l-tiles) -----------------------
            with ExitStack() as p8:
                f1_psp = p8.enter_context(
                    tc.tile_pool(name="f1_ps", bufs=2, space="PSUM"))
                o_psp = p8.enter_context(
                    tc.tile_pool(name="o_ps", bufs=1, space="PSUM"))
                w1_pool = p8.enter_context(tc.tile_pool(name="w1_pool", bufs=3))
                w2_pool = p8.enter_context(tc.tile_pool(name="w2_pool", bufs=3))
                ff1_sb = xtp.tile([128, NFF * LC], BF, name=f"ff1_{rep}", tag="ff1")
                for fq in range(8):
                    f1_ps = f1_psp.tile([128, 1024], F32, name=f"f1{rep}_{fq}",
                                        tag="f1")
                    zmm(f1_ps[:, 0:512], 512)
                    zmm(f1_ps[:, 512:1024], 512)
                    w14 = w1_pool.tile([128, 4 * NC_DM * 128], BF,
                                       name=f"w1s{rep}_{fq}", tag="w1s")
                    nc.sync.dma_start(
                        w14.rearrange("p (t x) -> p t x", t=4),
                        w1_in[fq * 4:(fq + 1) * 4].rearrange("t p x -> p t x"))
                    for f4 in range(4):
                        f = fq * 4 + f4
                        wo = f4 * NC_DM * 128
                        o = f1_ps[:, f4 * 256:(f4 + 1) * 256]
                        for c in range(NC_DM):
                            nc.tensor.matmul(
                                o, w14[:, wo + c * 128: wo + (c + 1) * 128],
                                xT_sb[:, c * LC:(c + 1) * LC],
                                start=False, stop=False, skip_group_check=True)
                        nc.tensor.matmul(
                            o, b1r_sb[0:1, f * 128:(f + 1) * 128],
                            ones_sb[0:1, 0:256],
                            start=False, stop=True, skip_group_check=True)
                    nc.scalar.activation(ff1_sb[:, fq * 1024:(fq + 1) * 1024],
                                         f1_ps[:], AF.Gelu)
                    if DBG and rep == 0 and fq == 0:
                        nc.gpsimd.dma_start(dbg_f1[:], ff1_sb[:, 0:1024])
                        nc.gpsimd.dma_start(dbg_xt[:], xT_sb[:, 0:2048])

                o_ps = [o_psp.tile([128, 1024], F32, name=f"o{rep}_{i}",
                                   tag=f"o{i}") for i in range(NLT)]
                for i in range(NLT):
                    zmm(o_ps[i][:, 0:512], 512)
                    zmm(o_ps[i][:, 512:1024], 512)
                    for hh in range(2):
                        nc.tensor.matmul(
                            o_ps[i][:, hh * 512:(hh + 1) * 512],
                            ones_sb[0:1, 0:128],
                            b2r_sb[0:1, hh * 512:(hh + 1) * 512],
                            start=False, stop=False, skip_group_check=True)
                for fq in range(8):
                    w24 = w2_pool.tile([128, 4 * DM], BF, name=f"w2s{rep}_{fq}",
                                       tag="w2s")
                    nc.sync.dma_start(
                        w24.rearrange("p (t x) -> p t x", t=4),
                        w2_in[fq * 4:(fq + 1) * 4].rearrange("t p x -> p t x"))
                    for f4 in range(4):
                        f = fq * 4 + f4
                        for i in range(NLT):
                            for hh in range(2):
                                nc.tensor.matmul(
                                    o_ps[i][:, hh * 512:(hh + 1) * 512],
                                    ff1_sb[:, f * LC + i * 128:
                                           f * LC + (i + 1) * 128],
                                    w24[:, f4 * DM + hh * 512:
                                        f4 * DM + (hh + 1) * 512],
                                    start=False, stop=(f == NFF - 1),
                                    skip_group_check=True)

                # ---------------- P9: residual + LN2 + out -----------------
                lp2 = p8.enter_context(tc.tile_pool(name="lp2", bufs=2))
                eps2 = eps_sb
                for i in range(NLT):
                    ff_bf = lp2.tile([128, 1024], BF, name=f"ffb{rep}_{i}", tag="ffb")
                    nc.scalar.copy(ff_bf[:], o_ps[i][:])
                    t1 = lp2.tile([128, 1024], BF, name=f"t1{rep}_{i}", tag="t1")
                    nc.vector.tensor_tensor(t1[:], xn_tiles[i][:], g1r_sb[:], AL.mult)
                    r2 = lp2.tile([128, 1024], BF, name=f"r2{rep}_{i}", tag="r2")
                    nc.vector.tensor_tensor(r2[:], t1[:], ff_bf[:], AL.add)

                    wv1 = lp2.tile([128, 1024], BF, name=f"wv1{rep}_{i}", tag="wv1")
                    rsum = lp2.tile([128, 1], F32, name=f"rs2{rep}_{i}", tag="rs2")
                    nc.scalar.activation(wv1[:], r2[:], AF.Copy, accum_out=rsum[:])
                    wv2 = lp2.tile([128, 1024], F32, name=f"wv2{rep}_{i}", tag="wv2")
                    ssq = lp2.tile([128, 1], F32, name=f"sq2{rep}_{i}", tag="sq2")
                    nc.scalar.activation(wv2[:], r2[:], AF.Square, accum_out=ssq[:])
                    mean = lp2.tile([128, 1], F32, name=f"mn2{rep}_{i}", tag="mn2")
                    nc.vector.tensor_scalar_mul(mean[:], rsum[:], 1.0 / DM)
                    nmean = lp2.tile([128, 1], F32, name=f"nm2{rep}_{i}", tag="nm2")
                    nc.vector.tensor_scalar_mul(nmean[:], rsum[:], -1.0 / DM)
                    exx = lp2.tile([128, 1], F32, name=f"ex2{rep}_{i}", tag="ex2")
                    nc.vector.tensor_scalar_mul(exx[:], ssq[:], 1.0 / DM)
                    m2 = lp2.tile([128, 1], F32, name=f"m22{rep}_{i}", tag="m22")
                    nc.vector.tensor_tensor(m2[:], mean[:], mean[:], AL.mult)
                    var = lp2.tile([128, 1], F32, name=f"vr2{rep}_{i}", tag="vr2")
                    nc.vector.tensor_tensor(var[:], exx[:], m2[:], AL.subtract)
                    std = lp2.tile([128, 1], F32, name=f"sd2{rep}_{i}", tag="sd2")
                    nc.scalar.activation(std[:], var[:], AF.Sqrt, bias=eps2[:])
                    rstd = lp2.tile([128, 1], F32, name=f"rv2{rep}_{i}", tag="rv2")
                    nc.vector.reciprocal(rstd[:], std[:])
                    xn2 = lp2.tile([128, 1024], BF, name=f"xn2{rep}_{i}", tag="xn2")
                    nc.vector.tensor_scalar(
                        out=xn2[:], in0=r2[:], scalar1=nmean[:], scalar2=rstd[:],
                        op0=AL.add, op1=AL.mult)
                    t3 = lp2.tile([128, 1024], BF, name=f"t3{rep}_{i}", tag="t3")
                    nc.vector.tensor_tensor(t3[:], xn2[:], g2r_sb[:], AL.mult)
                    y_bf = lp2.tile([128, 1024], BF, name=f"y{rep}_{i}", tag="y")
                    nc.vector.tensor_tensor(y_bf[:], t3[:], be2_sb[:], AL.add)
                    y_f = lp2.tile([128, 1024], F32, name=f"yf{rep}_{i}", tag="yf")
                    nc.scalar.copy(y_f[:], y_bf[:])
                    nc.sync.dma_start(out[i * 128:(i + 1) * 128, :], y_f[:])

    nc.compile()
    return nc


def _prep_core(src_c, tgt_c, mask_c, W):
    bf = ml_dtypes.bfloat16
    X = np.ascontiguousarray(tgt_c.reshape(LC * D, DM)).astype(np.float32)

    xt = np.zeros((NT, 128, CH, 128), dtype=np.float32)
    xt[:, :, :NC_DM, :] = X.reshape(NT, 128, NC_DM, 128).transpose(0, 3, 2, 1)
    xt[:, 0, NC_DM, :] = 1.0

    tgt = X.reshape(NT, 128, DM)

    st = np.zeros((128, CH, LC), dtype=np.float32)
    st[:, :NC_DM, :] = src_c.reshape(LC, NC_DM, 128).transpose(2, 1, 0)
    st[0, NC_DM, :] = 1.0

    Wp = np.zeros((CH * 128, DM), dtype=np.float32)
    Wp[:DM] = W["Wq"] / SCALE
    Wp[DM] = W["bq"] / SCALE
    wq = Wp.reshape(CH, 128, DM).transpose(1, 0, 2).reshape(128, CH * 1024)

    # wkz[p, (c*H+h)*128 + j] = Wk[c*128+j, h*64 + (p - 64*(h%2))] if p//64==h%2
    Wk = W["Wk"].reshape(NC_DM, 128, H, DH)  # [c, j, h, dh]
    wkz = np.zeros((128, NC_DM, H, 128), dtype=np.float32)
    for e in range(2):
        # heads with h%2==e sit at partitions 64e..64e+64
        wkz[64 * e:64 * (e + 1), :, e::2, :] = Wk[:, :, e::2, :].transpose(3, 0, 2, 1)
    wkz = wkz.reshape(128, NC_DM * H * 128)

    bk = W["bk"].reshape(H, DH)
    bkz = np.zeros((128, H), dtype=np.float32)
    for h in range(H):
        e = h % 2
        bkz[64 * e:64 * (e + 1), h] = bk[h]

    wv = np.ascontiguousarray(
        W["Wv"].reshape(NC_DM, 128, DM).transpose(1, 0, 2).reshape(128, NC_DM * 1024))

    mkf = np.zeros((1, NT * 128), dtype=np.float32)
    m = mask_c.reshape(NT, 4, D)  # [t, r//32, d]
    mkf[0] = np.repeat(m, 1, axis=1).transpose(0, 1, 2).reshape(NT, 128).reshape(-1)

    srcf = src_c + W["bv"][None, :]

    W1p = W["g1"][:, None] * W["W1"]  # fold LN1 gain
    w1p = np.ascontiguousarray(
        W1p.reshape(NC_DM, 128, NFF, 128).transpose(2, 1, 0, 3)
        .reshape(NFF, 128, NC_DM * 128))
    b1r = (W["beta1"] @ W["W1"] + W["b1"]).reshape(1, NFF * 128)
    w2p = np.ascontiguousarray(W["W2"].reshape(NFF, 128, DM))
    b2r = (W["b2"] + W["beta1"]).reshape(1, DM)

    return {
        "xt": xt.reshape(NT, 128, CH * 128).astype(bf),
        "tgt": tgt.astype(bf),
        "st": st.reshape(128, CH * LC).astype(bf),
        "wq": wq.astype(bf),
        "wkz": wkz.astype(bf),
        "bkz": bkz.astype(bf),
        "wv": wv.astype(bf),
        "mkf": mkf.astype(bf),
        "srcf": srcf.astype(bf),
        "w1p": w1p.astype(bf),
        "b1r": b1r.astype(bf),
        "w2p": w2p.astype(bf),
        "b2r": b2r.astype(bf),
        "g1r": np.tile(W["g1"], (128, 1)).astype(bf),
        "g2r": np.tile(W["g2"], (128, 1)).astype(bf),
        "be2r": np.tile(W["beta2"], (128, 1)).astype(bf),
    }


def make_in_maps(**inputs):
    inp = {k: np.asarray(v) for k, v in inputs.items()}
    W = {k: inp[k] for k in ("Wq", "bq", "Wk", "bk", "Wv", "bv", "W1", "b1",
                             "W2", "b2", "g1", "beta1", "g2", "beta2")}
    in_maps = []
    for c in range(NCORES):
        sl = slice(c * LC, (c + 1) * LC)
        in_maps.append(
            _prep_core(inp["src"][sl], inp["target"][sl], inp["attn_mask"][sl], W))
    return in_maps


def get_nc(repeat=1):
    key = ("nc", repeat)
    if key not in _CACHE:
        _CACHE[key] = _build_nc(repeat)
    return _CACHE[key]


def kernel(**inputs) -> np.ndarray:
    nc = get_nc()
    in_maps = make_in_maps(**inputs)
    res = run_bass_kernel_spmd(nc, in_maps, core_ids=list(range(NCORES)))
    return np.concatenate([res.results[c]["out"] for c in range(NCORES)], axis=0)


if __name__ == "__main__":
    import reference

    inputs = {k: np.asarray(v) for k, v in reference.setup_inputs().items()}
    got = kernel(**inputs)
    exp = np.asarray(reference.reference(**inputs))
    err = np.abs(got - exp).max() / np.abs(exp).max()
    print("Relative error:", err)
